# revision 1
# baseline (speedup 1.0000x reference)
"""Trainium2 Bass kernel for DenseGatPerfPlayerModel (2-layer masked GAT + MLP head).

Strategy (8 NeuronCores, data-parallel over batch B=32 -> G=4 graphs/core):
  - All per-graph tensors kept feature-major ("transposed", [feat, node]) so every
    projection is a single PE matmul with weights as lhsT.
  - Attention scores computed directly in [m(source), n(dest)] layout:
      sT[m,n] = (k[m]/4) . q[n]  via  matmul(lhsT=k_t[h, m-chunk], rhs=q_t[h]).
    Softmax runs over m. Scores are O(1) (measured range [-1.14, 0.9]) so
    exp needs no max-subtraction; softmax denominator is obtained for free by
    augmenting v with a ones-column and normalization is applied after the
    o-matmul (valid since softmax(s)=exp(s)*adj / sum(exp(s)*adj)).
  - PE operands with K<=32 must sit at partition base {0,32,64} (quadrant
    tile_position), so heads live at 32-partition stride, 3 per group tile:
    head h -> (group h//3, base 32*(h%3)), groups of (3,3,2) heads.
  - Mask: p = exp(sT) * adjT  (adj is 0/1; bf16 exact for the mask, and bf16 p/v
    keeps DVE in 2x mode; validated max rel err 1.2e-3 end-to-end).
  - Layer 2 is computed ONLY at the query node (the model head gathers a single
    node per graph), collapsing its [N,N] attention to [N,1].
  - Final 3-layer MLP computed on-device for all 4 graphs at once.

Host-side work is limited to data marshaling: sharding over cores, transposing
adj/node_features into the device layout, folding biases/scale into weight
matrices, and building one-hot/query-row helper vectors from query_idxs.
"""

import numpy as np

B, N = 32, 512
G = 4  # graphs per core
NCORES = 8
H, DH, DO, DLIN = 8, 16, 16, 64
DIN, DINIT = 16, 64
SCALE = 1999853.335557038
P = 128
MC = N // P  # 4 m-chunks per graph
NG = 3  # head groups: heads (0,1,2), (3,4,5), (6,7) at 32-partition stride
GSZ = [3, 3, 2]
HMAP = [(h // 3, h % 3) for h in range(H)]  # head -> (group, position)


def _build_nc():
    from contextlib import ExitStack

    import concourse.mybir as mybir
    import concourse.tile as tile
    from concourse import bacc

    f32 = mybir.dt.float32
    bf = mybir.dt.bfloat16
    AF = mybir.ActivationFunctionType
    ALU = mybir.AluOpType

    nc = bacc.Bacc()

    # ---- DRAM parameters (per-core shard) ----
    nf_d = nc.declare_dram_parameter("nf", [G, DIN + 1, N], f32, isOutput=False)
    adjT_d = nc.declare_dram_parameter("adjT", [G, N, N], f32, isOutput=False)
    adjq_d = nc.declare_dram_parameter("adjq", [G, P, MC], f32, isOutput=False)
    oh_d = nc.declare_dram_parameter("onehot", [G, P, MC], f32, isOutput=False)
    w_specs = {"Wi": [DIN + 1, DINIT]}
    for l, d in ((0, DINIT), (1, DLIN)):
        for grp in range(NG):
            w_specs[f"Wq{l}_{grp}"] = [d + 1, P]
            w_specs[f"Wk{l}_{grp}"] = [d + 1, P]
            w_specs[f"Wl{l}_{grp}"] = [P, DLIN]
        w_specs[f"Wv{l}"] = [d + 1, H * DO]
        w_specs[f"bl{l}"] = [DLIN, 1]
    for grp in range(NG):
        w_specs[f"E{grp}"] = [H, P]
        w_specs[f"Sel{grp}"] = [32 * GSZ[grp], H]
    w_specs.update({
        "I64p": [DLIN + 1, DLIN],
        "Wf0": [2 * DLIN, 128], "bf0": [128, 1],
        "Wf1": [128, 64], "bf1": [64, 1],
        "Wf2": [64, 1], "bf2": [1, 1],
    })
    w_d = {k: nc.declare_dram_parameter(k, s, f32, isOutput=False) for k, s in w_specs.items()}
    out_d = nc.declare_dram_parameter("out", [1, G], f32, isOutput=True)

    with tile.TileContext(nc) as tc, ExitStack() as ctx:
        wpool = ctx.enter_context(tc.tile_pool(name="w", bufs=1))
        gpool = ctx.enter_context(tc.tile_pool(name="g", bufs=2))
        stream = ctx.enter_context(tc.tile_pool(name="stream", bufs=3))
        work = ctx.enter_context(tc.tile_pool(name="work", bufs=3))
        persist = ctx.enter_context(tc.tile_pool(name="persist", bufs=1))
        ps_s = ctx.enter_context(tc.tile_pool(name="ps_s", bufs=2, space="PSUM"))
        ps_o = ctx.enter_context(tc.tile_pool(name="ps_o", bufs=1, space="PSUM"))
        ps_m = ctx.enter_context(tc.tile_pool(name="ps_m", bufs=1, space="PSUM"))

        # ---- weights to SBUF ----
        W = {}
        for k in w_specs:
            W[k] = wpool.tile(w_specs[k], f32, tag=f"w_{k}", name=f"w_{k}")
            nc.sync.dma_start(W[k][:], w_d[k][:])

        feat_sb = persist.tile([P, G], f32)  # [x1_q; x2_q] per graph column
        out_sb = persist.tile([1, G], f32)

        def elu_from_psum(dst, src, bias, p, f, tg):
            # dst = elu(src + bias); src is a PSUM AP [p, f]
            e = work.tile([p, f], f32, tag=f"elu_e_{tg}")
            r = work.tile([p, f], f32, tag=f"elu_r_{tg}")
            nc.scalar.activation(e[:], src, AF.Exp, bias=bias)
            nc.vector.tensor_scalar(e[:], e[:], 1.0, 0.0, ALU.subtract, ALU.min)
            nc.scalar.activation(r[:], src, AF.Relu, bias=bias)
            nc.vector.tensor_add(dst, e[:], r[:])

        for g in range(G):
            # ---------- load graph data ----------
            nf_sb = gpool.tile([DIN + 1, N], f32, tag="nf")
            nc.sync.dma_start(nf_sb[:], nf_d[g])
            adjb = gpool.tile([P, MC, N], bf, tag="adjb")  # bf16 adjT chunks
            for mc in range(MC):
                aj = stream.tile([P, N], f32, tag="ajf32")
                nc.sync.dma_start(aj[:], adjT_d[g, mc * P:(mc + 1) * P, :])
                nc.gpsimd.tensor_copy(adjb[:, mc, :], aj[:])
            adjq_sb = gpool.tile([P, MC], f32, tag="adjq")
            nc.sync.dma_start(adjq_sb[:], adjq_d[g])
            oh_sb = gpool.tile([P, MC], f32, tag="oh")
            nc.sync.dma_start(oh_sb[:], oh_d[g])

            # ---------- x0 = elu(nf @ Wi) ----------
            x0 = gpool.tile([DINIT + 1, N], f32, tag="x0")
            x0_ps = ps_m.tile([DINIT, N], f32, tag="m")
            nc.tensor.matmul(x0_ps[:], W["Wi"][:], nf_sb[:], start=True, stop=True)
            elu_from_psum(x0[0:DINIT, :], x0_ps[:], 0.0, DINIT, N, "x")
            nc.vector.memset(x0[DINIT:DINIT + 1, :], 1.0)

            def attn_layer1(x_aug, l, x1_dst):
                # q_t, k_t per head-group: [128, N], heads at 32-stride
                qt, kt = [], []
                for grp in range(NG):
                    for lst, wn in ((qt, f"Wq{l}_{grp}"), (kt, f"Wk{l}_{grp}")):
                        pr = ps_m.tile([P, N], f32, tag="m")
                        nc.tensor.matmul(pr[:], W[wn][:], x_aug[:], start=True, stop=True)
                        t = gpool.tile([P, N], f32, tag=f"qk_{wn}", name=f"t_{wn}")
                        nc.any.tensor_copy(t[:], pr[:])
                        lst.append(t)
                # v (node-major) + ones column per head, bf16
                vsb = gpool.tile([P, MC, H, 32], bf, tag="v1")
                nc.gpsimd.memset(vsb[:], 0.0)
                nc.vector.memset(vsb[:, :, :, DO:DO + 1], 1.0)
                for mc in range(MC):
                    vp = ps_m.tile([P, H * DO], f32, tag="m")
                    nc.tensor.matmul(vp[:], x_aug[:, mc * P:(mc + 1) * P], W[f"Wv{l}"][:],
                                     start=True, stop=True)
                    nc.any.tensor_copy(vsb[:, mc, :, 0:DO],
                                       vp.rearrange("p (h e) -> p h e", h=H))
                o_ps = [ps_o.tile([P, N], f32, tag=f"o{grp}", name=f"o{grp}")
                        for grp in range(NG)]
                # pairs drawn from different groups: a PSUM bank may hold only
                # one open accumulation group, and each head's o accumulates
                # over all 4 m-chunks
                for ha, hb in ((0, 3), (1, 4), (2, 6), (5, 7)):
                    for mc in range(MC):
                        sp = ps_s.tile([P, 2, N], f32, tag="s")
                        for r, h in enumerate((ha, hb)):
                            grp, pos = HMAP[h]
                            nc.tensor.matmul(
                                sp[:, r, :],
                                kt[grp][32 * pos:32 * pos + DH, mc * P:(mc + 1) * P],
                                qt[grp][32 * pos:32 * pos + DH, :],
                                start=True, stop=True)
                        ex = work.tile([P, 2, N], bf, tag="ex")
                        nc.scalar.activation(ex[:], sp[:], AF.Exp)
                        pm = work.tile([P, 2, N], bf, tag="pm")
                        nc.vector.tensor_tensor(
                            pm[:], ex[:],
                            adjb[:, mc, None, :].to_broadcast((P, 2, N)),
                            ALU.mult)
                        for r, h in enumerate((ha, hb)):
                            grp, pos = HMAP[h]
                            nc.tensor.matmul(
                                o_ps[grp][32 * pos:32 * pos + 32, :],
                                vsb[:, mc, h, :], pm[:, r, :],
                                start=(mc == 0), stop=(mc == MC - 1))
                # move o to SBUF (frees PSUM banks; DMA cannot read PSUM), then
                # gather denominators (rows 32*pos+DO, stride 32) via SBUF DMA
                o_sb = []
                for grp in range(NG):
                    t = work.tile([P, N], f32, tag=f"osb{grp}", name=f"osb{grp}")
                    nc.any.tensor_copy(t[0:32 * GSZ[grp], :],
                                       o_ps[grp][0:32 * GSZ[grp], :])
                    o_sb.append(t)
                den_ps = ps_m.tile([H, N], f32, tag="m")
                for grp in range(NG):
                    nc.tensor.matmul(den_ps[:], W[f"Sel{grp}"][:],
                                     o_sb[grp][0:32 * GSZ[grp], :],
                                     start=(grp == 0), stop=(grp == NG - 1))
                rec = work.tile([H, N], f32, tag="rec")
                nc.vector.reciprocal(rec[:], den_ps[:])
                scrs = []
                for grp in range(NG):
                    d_ps = ps_m.tile([P, N], f32, tag="m")
                    nc.tensor.matmul(d_ps[:], W[f"E{grp}"][:], rec[:], start=True, stop=True)
                    d_sb = work.tile([P, N], f32, tag=f"d{grp}", name=f"d{grp}")
                    nc.any.tensor_copy(d_sb[:], d_ps[:])
                    scr = work.tile([P, N], f32, tag=f"scr{grp}", name=f"scr{grp}")
                    r_g = 32 * GSZ[grp]
                    nc.vector.tensor_tensor(scr[0:r_g, :], o_sb[grp][0:r_g, :],
                                            d_sb[0:r_g, :], ALU.mult)
                    scrs.append(scr)
                x1_ps = ps_m.tile([DLIN, N], f32, tag="m")
                for grp in range(NG):
                    nc.tensor.matmul(x1_ps[:], W[f"Wl{l}_{grp}"][0:32 * GSZ[grp], :],
                                     scrs[grp][0:32 * GSZ[grp], :],
                                     start=(grp == 0), stop=(grp == NG - 1))
                elu_from_psum(x1_dst[0:DLIN, :], x1_ps[:], W[f"bl{l}"][:], DLIN, N, "x")

            x1 = gpool.tile([DLIN + 1, N], f32, tag="x1")
            attn_layer1(x0, 0, x1)
            nc.vector.memset(x1[DLIN:DLIN + 1, :], 1.0)

            # ---------- x1 in node-major layout, and x1 at the query node ----------
            x1nd = gpool.tile([P, MC, DLIN], f32, tag="x1nd")
            for mc in range(MC):
                ndp = ps_m.tile([P, DLIN], f32, tag="m")
                nc.tensor.matmul(ndp[:], x1[:, mc * P:(mc + 1) * P], W["I64p"][:],
                                 start=True, stop=True)
                nc.any.tensor_copy(x1nd[:, mc, :], ndp[:])
            x1q_ps = ps_m.tile([DLIN, 1], f32, tag="m")
            for mc in range(MC):
                nc.tensor.matmul(x1q_ps[:], x1nd[:, mc, :], oh_sb[:, mc:mc + 1],
                                 start=(mc == 0), stop=(mc == MC - 1))
            nc.any.tensor_copy(feat_sb[0:DLIN, g:g + 1], x1q_ps[:])
            x1qa = gpool.tile([DLIN + 1, 1], f32, tag="x1qa")
            nc.any.tensor_copy(x1qa[0:DLIN, :], x1q_ps[:])
            nc.vector.memset(x1qa[DLIN:DLIN + 1, :], 1.0)

            # ---------- layer 2 (query row only) ----------
            # q2 block-diagonal [128, GSZ] per group; k2t padded per group
            q2bd, k2t = [], []
            for grp in range(NG):
                q2_ps = ps_m.tile([P, 1], f32, tag="m")
                nc.tensor.matmul(q2_ps[:], W[f"Wq1_{grp}"][:], x1qa[:],
                                 start=True, stop=True)
                qb = gpool.tile([P, 3], f32, tag=f"q2bd{grp}", name=f"q2bd{grp}")
                nc.vector.memset(qb[:], 0.0)
                for pos in range(GSZ[grp]):
                    nc.any.tensor_copy(qb[32 * pos:32 * pos + DH, pos:pos + 1],
                                       q2_ps[32 * pos:32 * pos + DH, :])
                q2bd.append(qb)
                k2_ps = ps_m.tile([P, N], f32, tag="m")
                nc.tensor.matmul(k2_ps[:], W[f"Wk1_{grp}"][:], x1[:],
                                 start=True, stop=True)
                kb = gpool.tile([P, N], f32, tag=f"k2t{grp}", name=f"k2t{grp}")
                nc.any.tensor_copy(kb[:], k2_ps[:])
                k2t.append(kb)
            v2sb = gpool.tile([P, MC, H, 32], bf, tag="v2")
            nc.gpsimd.memset(v2sb[:], 0.0)
            nc.vector.memset(v2sb[:, :, :, DO:DO + 1], 1.0)
            for mc in range(MC):
                vp = ps_m.tile([P, H * DO], f32, tag="m")
                nc.tensor.matmul(vp[:], x1[:, mc * P:(mc + 1) * P], W["Wv1"][:],
                                 start=True, stop=True)
                nc.any.tensor_copy(v2sb[:, mc, :, 0:DO],
                                   vp.rearrange("p (h e) -> p h e", h=H))
                # fold the adjacency mask of the query row into v2 (incl. ones col)
                nc.vector.tensor_scalar_mul(
                    v2sb[:, mc, :, :], v2sb[:, mc, :, :], adjq_sb[:, mc:mc + 1])
            o2 = [persist.tile([P, 1], f32, tag=f"o2_{grp}", name=f"o2_{grp}")
                  for grp in range(NG)]
            for mc in range(MC):
                s2p = ps_m.tile([P, H], f32, tag="m")
                for grp in range(NG):
                    nc.tensor.matmul(s2p[:, 3 * grp:3 * grp + GSZ[grp]],
                                     k2t[grp][:, mc * P:(mc + 1) * P],
                                     q2bd[grp][:, 0:GSZ[grp]],
                                     start=True, stop=True)
                ex2 = work.tile([P, H], bf, tag="ex2")
                nc.scalar.activation(ex2[:], s2p[:], AF.Exp)
                for grp in range(NG):
                    o2p = ps_m.tile([P, 1], f32, tag="m", name=f"o2p{grp}")
                    r_g = 32 * GSZ[grp]
                    for pos in range(GSZ[grp]):
                        h = 3 * grp + pos
                        nc.tensor.matmul(o2p[32 * pos:32 * pos + 32, :],
                                         v2sb[:, mc, h, :], ex2[:, h:h + 1],
                                         start=True, stop=True)
                    if mc == 0:
                        nc.any.tensor_copy(o2[grp][0:r_g, :], o2p[0:r_g, :])
                    else:
                        nc.vector.tensor_add(o2[grp][0:r_g, :], o2[grp][0:r_g, :],
                                             o2p[0:r_g, :])
            den2_ps = ps_m.tile([H, 1], f32, tag="m")
            for grp in range(NG):
                nc.tensor.matmul(den2_ps[:], W[f"Sel{grp}"][:],
                                 o2[grp][0:32 * GSZ[grp], :],
                                 start=(grp == 0), stop=(grp == NG - 1))
            rec2 = work.tile([H, 1], f32, tag="rec2")
            nc.vector.reciprocal(rec2[:], den2_ps[:])
            scr2s = []
            for grp in range(NG):
                d2_ps = ps_m.tile([P, 1], f32, tag="m")
                nc.tensor.matmul(d2_ps[:], W[f"E{grp}"][:], rec2[:], start=True, stop=True)
                d2 = work.tile([P, 1], f32, tag=f"d2_{grp}", name=f"d2_{grp}")
                nc.any.tensor_copy(d2[:], d2_ps[:])
                scr2 = work.tile([P, 1], f32, tag=f"scr2_{grp}", name=f"scr2_{grp}")
                r_g = 32 * GSZ[grp]
                nc.vector.tensor_tensor(scr2[0:r_g, :], o2[grp][0:r_g, :],
                                        d2[0:r_g, :], ALU.mult)
                scr2s.append(scr2)
            x2_ps = ps_m.tile([DLIN, 1], f32, tag="m")
            for grp in range(NG):
                nc.tensor.matmul(x2_ps[:], W[f"Wl1_{grp}"][0:32 * GSZ[grp], :],
                                 scr2s[grp][0:32 * GSZ[grp], :],
                                 start=(grp == 0), stop=(grp == NG - 1))
            elu_from_psum(feat_sb[DLIN:2 * DLIN, g:g + 1], x2_ps[:],
                          W["bl1"][:], DLIN, 1, "q")

        # ---------- MLP head over all graphs ----------
        h1_ps = ps_m.tile([128, G], f32, tag="m")
        nc.tensor.matmul(h1_ps[:], W["Wf0"][:], feat_sb[:], start=True, stop=True)
        h1 = persist.tile([128, G], f32, tag="h1")
        elu_from_psum(h1[:], h1_ps[:], W["bf0"][:], 128, G, "m1")
        h2_ps = ps_m.tile([64, G], f32, tag="m")
        nc.tensor.matmul(h2_ps[:], W["Wf1"][:], h1[:], start=True, stop=True)
        h2 = persist.tile([64, G], f32, tag="h2")
        elu_from_psum(h2[:], h2_ps[:], W["bf1"][:], 64, G, "m2")
        h3_ps = ps_m.tile([1, G], f32, tag="m")
        nc.tensor.matmul(h3_ps[:], W["Wf2"][:], h2[:], start=True, stop=True)
        elu_from_psum(out_sb[:], h3_ps[:], W["bf2"][:], 1, G, "m3")
        nc.vector.tensor_scalar_mul(out_sb[:], out_sb[:], float(SCALE))
        nc.sync.dma_start(out_d[:], out_sb[:])

    nc.compile()
    return nc


def _prep_core_inputs(inputs, core):
    """Marshal one core's shard (graphs core*G .. core*G+G-1) into device layout."""
    f32 = np.float32
    sl = slice(core * G, (core + 1) * G)
    nf = np.asarray(inputs["node_features"], f32)[sl]     # [G, N, DIN]
    adj = np.asarray(inputs["adj"], f32)[sl]              # [G, N, N]
    masks = np.asarray(inputs["masks"], f32)[sl]          # [G, N]
    qidx = np.asarray(inputs["query_idxs"])[sl]           # [G]

    nf_aug = np.concatenate(
        [np.transpose(nf, (0, 2, 1)), np.ones((G, 1, N), f32)], axis=1)
    adjT = ((np.transpose(adj, (0, 2, 1)) > 0) & (masks[:, :, None] > 0)).astype(f32)
    adjq = np.stack([(adj[g, qidx[g]] > 0) & (masks[g] > 0) for g in range(G)])
    adjq = adjq.astype(f32).reshape(G, MC, P).transpose(0, 2, 1).copy()
    onehot = np.zeros((G, N), f32)
    onehot[np.arange(G), qidx] = 1.0
    onehot = onehot.reshape(G, MC, P).transpose(0, 2, 1).copy()
    return {
        "nf": np.ascontiguousarray(nf_aug),
        "adjT": np.ascontiguousarray(adjT),
        "adjq": np.ascontiguousarray(adjq),
        "onehot": np.ascontiguousarray(onehot),
    }


def _prep_weights(inputs):
    f32 = np.float32
    w = {}

    def aug(Wm, bv):  # stack bias as extra contraction row
        return np.concatenate([np.asarray(Wm, f32).reshape(Wm.shape[0], -1),
                               np.asarray(bv, f32).reshape(1, -1)], axis=0)

    def pad3(Wa):  # [d, H*DH] he-compact -> NG x [d, 128] group-padded (32-stride)
        outs = []
        for grp in range(NG):
            Om = np.zeros((Wa.shape[0], P), f32)
            for pos in range(GSZ[grp]):
                h = 3 * grp + pos
                Om[:, 32 * pos:32 * pos + DH] = Wa[:, DH * h:DH * (h + 1)]
            outs.append(Om)
        return outs

    w["Wi"] = aug(inputs["W_init"], inputs["b_init"])
    for l in range(2):
        s = 1.0 / np.sqrt(DH)
        for grp, Om in enumerate(pad3(aug(inputs[f"Wq{l}"], inputs[f"bq{l}"]))):
            w[f"Wq{l}_{grp}"] = Om
        for grp, Om in enumerate(pad3(aug(np.asarray(inputs[f"Wk{l}"], f32) * s,
                                          np.asarray(inputs[f"bk{l}"], f32) * s))):
            w[f"Wk{l}_{grp}"] = Om
        w[f"Wv{l}"] = aug(inputs[f"Wv{l}"], inputs[f"bv{l}"])
        Wl = np.asarray(inputs[f"Wl{l}"], f32)  # [H*DO, DLIN]
        for grp in range(NG):
            Wlp = np.zeros((P, DLIN), f32)
            for pos in range(GSZ[grp]):
                h = 3 * grp + pos
                Wlp[32 * pos:32 * pos + DO] = Wl[DO * h:DO * (h + 1)]
            w[f"Wl{l}_{grp}"] = Wlp
        w[f"bl{l}"] = np.asarray(inputs[f"bl{l}"], f32).reshape(DLIN, 1)
    for grp in range(NG):
        E = np.zeros((H, P), f32)
        Sel = np.zeros((32 * GSZ[grp], H), f32)
        for pos in range(GSZ[grp]):
            E[3 * grp + pos, 32 * pos:32 * pos + DO + 1] = 1.0
            Sel[32 * pos + DO, 3 * grp + pos] = 1.0
        w[f"E{grp}"] = E
        w[f"Sel{grp}"] = Sel
    w["I64p"] = np.concatenate([np.eye(DLIN, dtype=f32),
                                np.zeros((1, DLIN), f32)], axis=0)
    for j, pdim in ((0, 128), (1, 64), (2, 1)):
        w[f"Wf{j}"] = np.asarray(inputs[f"Wf{j}"], f32)
        w[f"bf{j}"] = np.asarray(inputs[f"bf{j}"], f32).reshape(pdim, 1)
    return w


def kernel(**inputs) -> np.ndarray:
    from concourse.bass_utils import run_bass_kernel_spmd

    nc = _build_nc()
    w = _prep_weights(inputs)
    in_maps = []
    for core in range(NCORES):
        m = _prep_core_inputs(inputs, core)
        m.update(w)
        in_maps.append(m)
    res = run_bass_kernel_spmd(nc, in_maps, list(range(NCORES)))
    out = np.concatenate([res.results[i]["out"][0] for i in range(NCORES)])
    return out.astype(np.float32).reshape(B, 1)



# revision 8
# speedup vs baseline: 2.2585x; 2.2585x over previous
"""Trainium2 Bass kernel for DenseGatPerfPlayerModel (2-layer masked GAT + MLP head).

Strategy (8 NeuronCores, data-parallel over batch B=32 -> G=4 graphs/core):
  - All matmul operands in bf16: fp32 matmuls run the PE in LOW_HIGH double-pass
    mode (~1.1us per 512-col matmul vs ~0.25us bf16) and disable fast weight
    load. Adjacency/masks/features are exact or near-exact in bf16.
  - Per-graph tensors feature-major ([feat, node]); scores computed in
    [m(source), n(dest)] layout, softmax over m; exp without max-subtraction
    (scores O(1)); denominator via ones-column in v; mask applied as
    pm = exp(sT) * adjT (bf16).
  - PE quadrant rule: K<=32 operands at partition base {0,32,64}; heads at
    32-partition stride, groups of (3,3,2).
  - Emission is phase-interleaved across graphs so every engine's in-order
    queue stays full: A(g)=load+proj, B(g)=attention inner loop (software
    pipelined: score-mms of iter i+1 are emitted before o-mms of iter i),
    C(g)=softmax finish + Wl + elu, D(g)=layer 2 at the query node only.
    Round order: A0, [B(g), D(g-1), C(g), A(g+1)] for g=0..3, D3, MLP.
  - Engine balance: exp on ACT (bottleneck, ~1.15us/tile), mask-mult on
    GpSimd (SBUF-only operands), PSUM->SBUF copies on DVE, relu-part of elu
    on DVE (max), single weight-blob DMA.

Host-side work is data marshaling only: shard over cores, device layouts,
bf16 casts, bias folding (ones-row augmentation), one-hot/query-row vectors.
"""

import numpy as np
import ml_dtypes

B, N = 32, 512
G = 4  # graphs per core
NCORES = 8
H, DH, DO, DLIN = 8, 16, 16, 64
DIN, DINIT = 16, 64
SCALE = 1999853.335557038
P = 128
MC = N // P  # 4 m-chunks per graph
NG = 3
GSZ = [3, 3, 2]
HMAP = [(h // 3, h % 3) for h in range(H)]
PAIRS = ((0, 3), (1, 4), (2, 6), (5, 7))  # cross-group head pairs

BF = ml_dtypes.bfloat16


def _blob_layout():
    """Column layout of the bf16 weight blob (all tensors at row 0)."""
    L = {}
    c = 0

    def add(name, rows, cols):
        nonlocal c
        L[name] = (rows, c, cols)
        c += cols

    add("Wi", DIN + 1, DINIT)
    for l in range(2):
        for grp in range(NG):
            add(f"Wq{l}_{grp}", 65, P)
            add(f"Wk{l}_{grp}", 65, P)
        add(f"Wv{l}", 65, H * DO)
        for grp in range(NG):
            add(f"Wl{l}_{grp}", 32 * GSZ[grp], DLIN)
    for grp in range(NG):
        add(f"E{grp}", H, P)
        add(f"Sel{grp}", 32 * GSZ[grp], H)
    add("I64p", DLIN + 1, DLIN)
    return L, c


def _f32_layout():
    F = {}
    c = 0

    def add(name, rows, cols):
        nonlocal c
        F[name] = (rows, c, cols)
        c += cols

    add("Wf0", 2 * DLIN, 128)
    add("Wf1", 128, 64)
    add("Wf2", 64, 1)
    add("bl0", DLIN, 1)
    add("bl1", DLIN, 1)
    add("bf0", 128, 1)
    add("bf1", 64, 1)
    add("bf2", 1, 1)
    return F, c


BLOB_L, WCOLS = _blob_layout()
F32_L, FCOLS = _f32_layout()


def _build_nc():
    from contextlib import ExitStack

    import concourse.mybir as mybir
    import concourse.tile as tile
    from concourse import bacc

    f32 = mybir.dt.float32
    bf = mybir.dt.bfloat16
    AF = mybir.ActivationFunctionType
    ALU = mybir.AluOpType

    nc = bacc.Bacc()

    nf_d = nc.declare_dram_parameter("nf", [DIN + 1, G, N], bf, isOutput=False)
    adj_d = nc.declare_dram_parameter("adjT", [G, P, MC, N], bf, isOutput=False)
    aqoh_d = nc.declare_dram_parameter("aqoh", [P, G, 2, MC], bf, isOutput=False)
    aq32_d = nc.declare_dram_parameter("aq32", [P, G, MC], f32, isOutput=False)
    wb_d = nc.declare_dram_parameter("wb", [P, WCOLS], bf, isOutput=False)
    wf_d = nc.declare_dram_parameter("wf", [P, FCOLS], f32, isOutput=False)
    out_d = nc.declare_dram_parameter("out", [1, G], f32, isOutput=True)

    with tile.TileContext(nc) as tc, ExitStack() as ctx:
        wpool = ctx.enter_context(tc.tile_pool(name="w", bufs=1))
        gpool = ctx.enter_context(tc.tile_pool(name="g", bufs=2))
        work = ctx.enter_context(tc.tile_pool(name="work", bufs=3))
        misc = ctx.enter_context(tc.tile_pool(name="misc", bufs=2))
        persist = ctx.enter_context(tc.tile_pool(name="persist", bufs=1))
        ps_s = ctx.enter_context(tc.tile_pool(name="ps_s", bufs=2, space="PSUM"))
        ps_o = ctx.enter_context(tc.tile_pool(name="ps_o", bufs=1, space="PSUM"))
        ps_m = ctx.enter_context(tc.tile_pool(name="ps_m", bufs=1, space="PSUM"))

        # ---- DMAs (all issued up front; transfers overlap compute) ----
        wb_sb = wpool.tile([P, WCOLS], bf)
        nc.sync.dma_start(wb_sb[:], wb_d[:])
        wf_sb = wpool.tile([P, FCOLS], f32)
        nc.sync.dma_start(wf_sb[:], wf_d[:])
        nfT = wpool.tile([DIN + 1, G, N], bf)
        nc.sync.dma_start(nfT[:], nf_d[:])
        aqoh = wpool.tile([P, G, 2, MC], bf)
        nc.sync.dma_start(aqoh[:], aqoh_d[:])
        aq32 = wpool.tile([P, G, MC], f32)
        nc.sync.dma_start(aq32[:], aq32_d[:])
        adjb = wpool.tile([P, G, MC, N], bf)
        for g_ in range(G):
            nc.sync.dma_start(adjb[:, g_, :, :], adj_d[g_])

        W = {k: wb_sb[0:r, c:c + n] for k, (r, c, n) in BLOB_L.items()}
        F = {k: wf_sb[0:r, c:c + n] for k, (r, c, n) in F32_L.items()}

        feat_sb = persist.tile([P, G], f32)
        out_sb = persist.tile([1, G], f32)

        def elu(dst, src_ps, bias, p, f, dt, tg):
            # dst = elu(src+bias) = min(exp(src+bias)-1, 0) + max(src+bias, 0)
            # ACT does only the exp pass (it is the global bottleneck).
            e = work.tile([p, f], dt, tag=f"elu_e_{tg}", name=f"elu_e_{tg}")
            nc.scalar.activation(e[:], src_ps, AF.Exp, bias=bias)
            nc.vector.tensor_scalar(e[:], e[:], 1.0, 0.0, ALU.subtract, ALU.min)
            r = work.tile([p, f], dt, tag=f"elu_r_{tg}", name=f"elu_r_{tg}")
            if isinstance(bias, float):
                nc.vector.tensor_scalar(r[:], src_ps, bias, 0.0, ALU.add, ALU.max)
            else:
                nc.vector.tensor_scalar(r[:], src_ps, bias, 0.0, ALU.add, ALU.max)
            nc.vector.tensor_add(dst, e[:], r[:])

        # per-graph state
        x0a = [None] * G
        qkt = [[None] * NG for _ in range(G)]
        vsb = [None] * G
        o_ps_g = [None] * G
        x1a = [None] * G

        def phaseA(g):
            x0ps = ps_m.tile([DINIT, N], f32, tag="m", name="x0ps")
            nc.tensor.matmul(x0ps[:], W["Wi"][:], nfT[:, g, :], start=True, stop=True)
            xa = gpool.tile([DINIT + 1, N], bf, tag="x0a", name="x0a")
            elu(xa[0:DINIT, :], x0ps[:], 0.0, DINIT, N, bf, "x0")
            nc.vector.memset(xa[DINIT:DINIT + 1, :], 1.0)
            x0a[g] = xa
            for grp in range(NG):
                sp = ps_s.tile([P, 2, N], f32, tag="s", name=f"qkp{grp}")
                nc.tensor.matmul(sp[:, 0, :], W[f"Wq0_{grp}"][:], xa[:],
                                 start=True, stop=True)
                nc.tensor.matmul(sp[:, 1, :], W[f"Wk0_{grp}"][:], xa[:],
                                 start=True, stop=True)
                qk = gpool.tile([P, 2, N], bf, tag=f"qk{grp}", name=f"qk{grp}")
                if grp == 1:
                    nc.scalar.copy(qk[:], sp[:])
                else:
                    nc.vector.tensor_copy(qk[:], sp[:])
                qkt[g][grp] = qk
            vps = ps_s.tile([P, MC, P], f32, tag="s", name="vps")
            for mc in range(MC):
                nc.tensor.matmul(vps[:, mc, :], xa[:, mc * P:(mc + 1) * P],
                                 W["Wv0"][:], start=True, stop=True)
            vt = gpool.tile([P, MC, H, 32], bf, tag="vsb", name="vsb")
            nc.gpsimd.memset(vt[:], 0.0)
            nc.vector.memset(vt[:, :, :, DO:DO + 1], 1.0)
            for mc in range(MC):
                nc.vector.tensor_copy(
                    vt[:, mc, :, 0:DO],
                    vps[:, mc, :].rearrange("p (h e) -> p h e", h=H))
            vsb[g] = vt

        def phaseB(g):
            o_ps = [ps_o.tile([P, N], f32, tag=f"o{grp}", name=f"o{grp}")
                    for grp in range(NG)]
            o_ps_g[g] = o_ps
            iters = [(pr, mc) for pr in PAIRS for mc in range(MC)]
            pend = None
            for idx in range(len(iters) + 1):
                if idx < len(iters):
                    (ha, hb), mc = iters[idx]
                    sp = ps_s.tile([P, 2, N], f32, tag="s", name="sp")
                    for r, h in enumerate((ha, hb)):
                        grp, pos = HMAP[h]
                        nc.tensor.matmul(
                            sp[:, r, :],
                            qkt[g][grp][32 * pos:32 * pos + DH, 1, mc * P:(mc + 1) * P],
                            qkt[g][grp][32 * pos:32 * pos + DH, 0, :],
                            start=True, stop=True)
                    ex = work.tile([P, 2, N], bf, tag="ex")
                    nc.scalar.activation(ex[:], sp[:], AF.Exp)
                    pm = work.tile([P, 2, N], bf, tag="pm")
                    nc.gpsimd.tensor_tensor(
                        pm[:], ex[:],
                        adjb[:, g, mc, None, :].to_broadcast((P, 2, N)),
                        ALU.mult)
                    cur = ((ha, hb), mc, pm)
                else:
                    cur = None
                if pend is not None:
                    (ha, hb), mc, pmp = pend
                    for r, h in enumerate((ha, hb)):
                        grp, pos = HMAP[h]
                        nc.tensor.matmul(
                            o_ps[grp][32 * pos:32 * pos + 32, :],
                            vsb[g][:, mc, h, :], pmp[:, r, :],
                            start=(mc == 0), stop=(mc == MC - 1))
                pend = cur

        def phaseC(g):
            o_ps = o_ps_g[g]
            osb = []
            for grp in range(NG):
                rg = 32 * GSZ[grp]
                t = gpool.tile([rg, N], bf, tag=f"osb{grp}", name=f"osb{grp}")
                if grp == 1:
                    nc.scalar.copy(t[:], o_ps[grp][0:rg, :])
                else:
                    nc.vector.tensor_copy(t[:], o_ps[grp][0:rg, :])
                osb.append(t)
            den = ps_m.tile([H, N], f32, tag="m", name="den")
            for grp in range(NG):
                nc.tensor.matmul(den[:], W[f"Sel{grp}"][:], osb[grp][:],
                                 start=(grp == 0), stop=(grp == NG - 1))
            rec = misc.tile([H, N], bf, tag="rec")
            with nc.allow_low_precision(reason="softmax denom O(1..128); bf16 ok"):
                nc.vector.reciprocal(rec[:], den[:])
            scrs = []
            for grp in range(NG):
                rg = 32 * GSZ[grp]
                dps = ps_m.tile([P, N], f32, tag="m", name="dps")
                nc.tensor.matmul(dps[:], W[f"E{grp}"][:], rec[:], start=True, stop=True)
                dsb = misc.tile([P, N], bf, tag="dsb")
                nc.vector.tensor_copy(dsb[:], dps[:])
                scr = misc.tile([rg, N], bf, tag=f"scr{grp}", name=f"scr{grp}")
                nc.vector.tensor_tensor(scr[:], osb[grp][:], dsb[0:rg, :], ALU.mult)
                scrs.append(scr)
            x1ps = ps_m.tile([DLIN, N], f32, tag="m", name="x1ps")
            for grp in range(NG):
                nc.tensor.matmul(x1ps[:], W[f"Wl0_{grp}"][:], scrs[grp][:],
                                 start=(grp == 0), stop=(grp == NG - 1))
            xa = gpool.tile([DLIN + 1, N], bf, tag="x1a", name="x1a")
            elu(xa[0:DLIN, :], x1ps[:], F["bl0"][:], DLIN, N, bf, "x1")
            nc.vector.memset(xa[DLIN:DLIN + 1, :], 1.0)
            x1a[g] = xa

        def phaseD(g):
            xa = x1a[g]
            # x1 node-major + gather at query node
            ndp = ps_m.tile([P, MC, DLIN], f32, tag="m", name="ndp")
            for mc in range(MC):
                nc.tensor.matmul(ndp[:, mc, :], xa[:, mc * P:(mc + 1) * P],
                                 W["I64p"][:], start=True, stop=True)
            x1nd = gpool.tile([P, MC, DLIN], bf, tag="x1nd", bufs=1, name="x1nd")
            nc.vector.tensor_copy(x1nd[:], ndp[:])
            x1qps = ps_m.tile([DLIN, 1], f32, tag="m", name="x1qps")
            for mc in range(MC):
                nc.tensor.matmul(x1qps[:], x1nd[:, mc, :],
                                 aqoh[:, g, 1, mc:mc + 1],
                                 start=(mc == 0), stop=(mc == MC - 1))
            nc.vector.tensor_copy(feat_sb[0:DLIN, g:g + 1], x1qps[:])
            x1qa = gpool.tile([DLIN + 1, 1], bf, tag="x1qa", bufs=1, name="x1qa")
            nc.vector.tensor_copy(x1qa[0:DLIN, :], x1qps[:])
            nc.vector.memset(x1qa[DLIN:DLIN + 1, :], 1.0)
            # layer-2 projections
            q2ps = ps_m.tile([P, NG], f32, tag="m", name="q2ps")
            for grp in range(NG):
                nc.tensor.matmul(q2ps[:, grp:grp + 1], W[f"Wq1_{grp}"][:], x1qa[:],
                                 start=True, stop=True)
            q2bd = []
            for grp in range(NG):
                qb = gpool.tile([P, 3], bf, tag=f"q2bd{grp}", bufs=1, name=f"q2bd{grp}")
                nc.vector.memset(qb[:], 0.0)
                for pos in range(GSZ[grp]):
                    nc.vector.tensor_copy(qb[32 * pos:32 * pos + DH, pos:pos + 1],
                                          q2ps[32 * pos:32 * pos + DH, grp:grp + 1])
                q2bd.append(qb)
            k2t = []
            for grp in range(NG):
                k2ps = ps_s.tile([P, N], f32, tag="s", name=f"k2ps{grp}")
                nc.tensor.matmul(k2ps[:], W[f"Wk1_{grp}"][:], xa[:],
                                 start=True, stop=True)
                kb = gpool.tile([P, N], bf, tag=f"k2t{grp}", bufs=1, name=f"k2t{grp}")
                if grp == 1:
                    nc.scalar.copy(kb[:], k2ps[:])
                else:
                    nc.vector.tensor_copy(kb[:], k2ps[:])
                k2t.append(kb)
            v2ps = ps_s.tile([P, MC, P], f32, tag="s", name="v2ps")
            for mc in range(MC):
                nc.tensor.matmul(v2ps[:, mc, :], xa[:, mc * P:(mc + 1) * P],
                                 W["Wv1"][:], start=True, stop=True)
            v2 = gpool.tile([P, MC, H, 32], bf, tag="v2sb", bufs=1, name="v2sb")
            nc.gpsimd.memset(v2[:], 0.0)
            nc.vector.memset(v2[:, :, :, DO:DO + 1], 1.0)
            for mc in range(MC):
                nc.vector.tensor_copy(
                    v2[:, mc, :, 0:DO],
                    v2ps[:, mc, :].rearrange("p (h e) -> p h e", h=H))
                # fold query-row adjacency mask (incl. ones col)
                nc.vector.tensor_scalar_mul(
                    v2[:, mc, :, :], v2[:, mc, :, :], aq32[:, g, mc:mc + 1])
            # attention at query node; o2 partials in PSUM columns (grp, mc)
            o2p = ps_m.tile([P, NG * MC], f32, tag="m", name="o2p")
            for mc in range(MC):
                s2p = ps_s.tile([P, H], f32, tag="s", name="s2p")
                for grp in range(NG):
                    nc.tensor.matmul(s2p[:, 3 * grp:3 * grp + GSZ[grp]],
                                     k2t[grp][:, mc * P:(mc + 1) * P],
                                     q2bd[grp][:, 0:GSZ[grp]],
                                     start=True, stop=True)
                ex2 = misc.tile([P, H], bf, tag="ex2")
                nc.scalar.activation(ex2[:], s2p[:], AF.Exp)
                for grp in range(NG):
                    for pos in range(GSZ[grp]):
                        h = 3 * grp + pos
                        nc.tensor.matmul(o2p[32 * pos:32 * pos + 32,
                                             grp * MC + mc:grp * MC + mc + 1],
                                         v2[:, mc, h, :], ex2[:, h:h + 1],
                                         start=True, stop=True)
            o2f = misc.tile([P, NG], f32, tag="o2f")
            nc.vector.tensor_reduce(o2f[:], o2p.rearrange("p (g m) -> p g m", g=NG),
                                    mybir.AxisListType.X, ALU.add)
            o2b = misc.tile([P, NG], bf, tag="o2b")
            nc.vector.tensor_copy(o2b[:], o2f[:])
            den2 = ps_m.tile([H, 1], f32, tag="m", name="den2")
            for grp in range(NG):
                nc.tensor.matmul(den2[:], W[f"Sel{grp}"][:],
                                 o2b[0:32 * GSZ[grp], grp:grp + 1],
                                 start=(grp == 0), stop=(grp == NG - 1))
            rec2 = misc.tile([H, 1], bf, tag="rec2")
            with nc.allow_low_precision(reason="softmax denom O(1..128); bf16 ok"):
                nc.vector.reciprocal(rec2[:], den2[:])
            d2ps = ps_m.tile([P, NG], f32, tag="m", name="d2ps")
            for grp in range(NG):
                nc.tensor.matmul(d2ps[:, grp:grp + 1], W[f"E{grp}"][:], rec2[:],
                                 start=True, stop=True)
            d2f = misc.tile([P, NG], f32, tag="d2f")
            nc.vector.tensor_copy(d2f[:], d2ps[:])
            scr2s = []
            for grp in range(NG):
                rg = 32 * GSZ[grp]
                scr2 = misc.tile([rg, 1], bf, tag=f"scr2_{grp}", name=f"scr2_{grp}")
                nc.vector.tensor_tensor(scr2[:], o2f[0:rg, grp:grp + 1],
                                        d2f[0:rg, grp:grp + 1], ALU.mult)
                scr2s.append(scr2)
            x2ps = ps_m.tile([DLIN, 1], f32, tag="m", name="x2ps")
            for grp in range(NG):
                nc.tensor.matmul(x2ps[:], W[f"Wl1_{grp}"][:], scr2s[grp][:],
                                 start=(grp == 0), stop=(grp == NG - 1))
            elu(feat_sb[DLIN:2 * DLIN, g:g + 1], x2ps[:], F["bl1"][:],
                DLIN, 1, f32, "x2")

        # ---- emission schedule ----
        phaseA(0)
        for g in range(G):
            phaseB(g)
            if g >= 1:
                phaseD(g - 1)
            phaseC(g)
            if g + 1 < G:
                phaseA(g + 1)
        phaseD(G - 1)

        # ---- MLP head over all graphs (fp32, tiny) ----
        h1ps = ps_m.tile([128, G], f32, tag="m", name="h1ps")
        nc.tensor.matmul(h1ps[:], F["Wf0"][:], feat_sb[:], start=True, stop=True)
        h1 = persist.tile([128, G], f32, tag="h1")
        elu(h1[:], h1ps[:], F["bf0"][:], 128, G, f32, "m1")
        h2ps = ps_m.tile([64, G], f32, tag="m", name="h2ps")
        nc.tensor.matmul(h2ps[:], F["Wf1"][:], h1[:], start=True, stop=True)
        h2 = persist.tile([64, G], f32, tag="h2")
        elu(h2[:], h2ps[:], F["bf1"][:], 64, G, f32, "m2")
        h3ps = ps_m.tile([1, G], f32, tag="m", name="h3ps")
        nc.tensor.matmul(h3ps[:], F["Wf2"][:], h2[:], start=True, stop=True)
        elu(out_sb[:], h3ps[:], F["bf2"][:], 1, G, f32, "m3")
        nc.vector.tensor_scalar_mul(out_sb[:], out_sb[:], float(SCALE))
        nc.sync.dma_start(out_d[:], out_sb[:])

    nc.compile()
    return nc


def _prep_weights(inputs):
    f32 = np.float32
    vals = {}

    def aug(Wm, bv):
        return np.concatenate([np.asarray(Wm, f32).reshape(Wm.shape[0], -1),
                               np.asarray(bv, f32).reshape(1, -1)], axis=0)

    def pad3(Wa):  # [d, H*DH] -> NG x [d, 128] group-padded (32-stride)
        outs = []
        for grp in range(NG):
            Om = np.zeros((Wa.shape[0], P), f32)
            for pos in range(GSZ[grp]):
                h = 3 * grp + pos
                Om[:, 32 * pos:32 * pos + DH] = Wa[:, DH * h:DH * (h + 1)]
            outs.append(Om)
        return outs

    vals["Wi"] = aug(inputs["W_init"], inputs["b_init"])
    for l in range(2):
        s = 1.0 / np.sqrt(DH)
        for grp, Om in enumerate(pad3(aug(inputs[f"Wq{l}"], inputs[f"bq{l}"]))):
            vals[f"Wq{l}_{grp}"] = Om
        for grp, Om in enumerate(pad3(aug(np.asarray(inputs[f"Wk{l}"], f32) * s,
                                          np.asarray(inputs[f"bk{l}"], f32) * s))):
            vals[f"Wk{l}_{grp}"] = Om
        vals[f"Wv{l}"] = aug(inputs[f"Wv{l}"], inputs[f"bv{l}"])
        Wl = np.asarray(inputs[f"Wl{l}"], f32)  # [H*DO, DLIN]
        for grp in range(NG):
            Wlp = np.zeros((32 * GSZ[grp], DLIN), f32)
            for pos in range(GSZ[grp]):
                h = 3 * grp + pos
                Wlp[32 * pos:32 * pos + DO] = Wl[DO * h:DO * (h + 1)]
            vals[f"Wl{l}_{grp}"] = Wlp
    for grp in range(NG):
        E = np.zeros((H, P), f32)
        Sel = np.zeros((32 * GSZ[grp], H), f32)
        for pos in range(GSZ[grp]):
            E[3 * grp + pos, 32 * pos:32 * pos + DO + 1] = 1.0
            Sel[32 * pos + DO, 3 * grp + pos] = 1.0
        vals[f"E{grp}"] = E
        vals[f"Sel{grp}"] = Sel
    vals["I64p"] = np.concatenate([np.eye(DLIN, dtype=f32),
                                   np.zeros((1, DLIN), f32)], axis=0)

    wb = np.zeros((P, WCOLS), f32)
    for k, (r, c, n) in BLOB_L.items():
        wb[0:r, c:c + n] = vals[k]

    fvals = {
        "Wf0": np.asarray(inputs["Wf0"], f32),
        "Wf1": np.asarray(inputs["Wf1"], f32),
        "Wf2": np.asarray(inputs["Wf2"], f32),
        "bl0": np.asarray(inputs["bl0"], f32).reshape(DLIN, 1),
        "bl1": np.asarray(inputs["bl1"], f32).reshape(DLIN, 1),
        "bf0": np.asarray(inputs["bf0"], f32).reshape(128, 1),
        "bf1": np.asarray(inputs["bf1"], f32).reshape(64, 1),
        "bf2": np.asarray(inputs["bf2"], f32).reshape(1, 1),
    }
    wf = np.zeros((P, FCOLS), f32)
    for k, (r, c, n) in F32_L.items():
        wf[0:r, c:c + n] = fvals[k]
    return {"wb": wb.astype(BF), "wf": wf}


def _prep_core_inputs(inputs, core):
    """Marshal one core's shard (graphs core*G .. core*G+G-1) into device layout."""
    f32 = np.float32
    sl = slice(core * G, (core + 1) * G)
    nf = np.asarray(inputs["node_features"], f32)[sl]     # [G, N, DIN]
    adj = np.asarray(inputs["adj"], f32)[sl]              # [G, N, N]
    masks = np.asarray(inputs["masks"], f32)[sl]          # [G, N]
    qidx = np.asarray(inputs["query_idxs"])[sl]           # [G]

    nfT = np.ones((DIN + 1, G, N), f32)
    nfT[0:DIN] = np.transpose(nf, (2, 0, 1))

    adjm = ((np.transpose(adj, (0, 2, 1)) > 0) & (masks[:, :, None] > 0)).astype(f32)
    # [G, N(m), N(n)] -> [G, 128(p), MC, N]: row mc*128+p -> (p, mc)
    adjdev = adjm.reshape(G, MC, P, N).transpose(0, 2, 1, 3)

    aqoh = np.zeros((P, G, 2, MC), f32)
    for g in range(G):
        aq = ((adj[g, qidx[g]] > 0) & (masks[g] > 0)).astype(f32)
        aqoh[:, g, 0, :] = aq.reshape(MC, P).T
        oh = np.zeros(N, f32)
        oh[qidx[g]] = 1.0
        aqoh[:, g, 1, :] = oh.reshape(MC, P).T
    return {
        "nf": np.ascontiguousarray(nfT).astype(BF),
        "adjT": np.ascontiguousarray(adjdev).astype(BF),
        "aqoh": np.ascontiguousarray(aqoh).astype(BF),
        "aq32": np.ascontiguousarray(aqoh[:, :, 0, :]),
    }


def kernel(**inputs) -> np.ndarray:
    from concourse.bass_utils import run_bass_kernel_spmd

    nc = _build_nc()
    w = _prep_weights(inputs)
    in_maps = []
    for core in range(NCORES):
        m = _prep_core_inputs(inputs, core)
        m.update(w)
        in_maps.append(m)
    res = run_bass_kernel_spmd(nc, in_maps, list(range(NCORES)))
    out = np.concatenate([res.results[i]["out"][0] for i in range(NCORES)])
    return out.astype(np.float32).reshape(B, 1)


# revision 15
# speedup vs baseline: 2.8522x; 1.2629x over previous
"""Trainium2 Bass kernel for DenseGatPerfPlayerModel (2-layer masked GAT + MLP head).

Strategy (8 NeuronCores, data-parallel over batch B=32 -> G=4 graphs/core):
  - All matmul operands in bf16: fp32 matmuls run the PE in LOW_HIGH double-pass
    mode (~1.1us per 512-col matmul vs ~0.25us bf16) and disable fast weight
    load. Adjacency/masks/features are exact or near-exact in bf16.
  - Per-graph tensors feature-major ([feat, node]); scores computed in
    [m(source), n(dest)] layout, softmax over m; exp without max-subtraction
    (scores O(1)); denominator via ones-column in v; mask applied as
    pm = exp(sT) * adjT (bf16).
  - PE quadrant rule: K<=32 operands at partition base {0,32,64}; heads at
    32-partition stride, groups of (3,3,2).
  - Emission is phase-interleaved across graphs so every engine's in-order
    queue stays full: A(g)=load+proj, B(g)=attention inner loop (software
    pipelined: score-mms of iter i+1 are emitted before o-mms of iter i),
    C(g)=softmax finish + Wl + elu, D(g)=layer 2 at the query node only.
    Round order: A0, [B(g), D(g-1), C(g), A(g+1)] for g=0..3, D3, MLP.
  - Engine balance: exp on ACT (bottleneck, ~1.15us/tile), mask-mult on
    GpSimd (SBUF-only operands), PSUM->SBUF copies on DVE, relu-part of elu
    on DVE (max), single weight-blob DMA.

Host-side work is data marshaling only: shard over cores, device layouts,
bf16 casts, bias folding (ones-row augmentation), one-hot/query-row vectors.
"""

import numpy as np
import ml_dtypes

B, N = 32, 512
G = 4  # graphs per core
NCORES = 8
H, DH, DO, DLIN = 8, 16, 16, 64
DIN, DINIT = 16, 64
SCALE = 1999853.335557038
P = 128
MC = N // P  # 4 m-chunks per graph
NG = 3
GSZ = [3, 3, 2]
HMAP = [(h // 3, h % 3) for h in range(H)]
PAIRS = ((0, 3), (1, 4), (2, 6), (5, 7))  # cross-group head pairs

BF = ml_dtypes.bfloat16


def _blob_layout():
    """Column layout of the bf16 weight blob (all tensors at row 0)."""
    L = {}
    c = 0

    def add(name, rows, cols):
        nonlocal c
        L[name] = (rows, c, cols)
        c += cols

    add("Wi", DIN + 1, DINIT)
    for l in range(2):
        for grp in range(NG):
            add(f"Wq{l}_{grp}", 65, P)
            add(f"Wk{l}_{grp}", 65, P)
        add(f"Wv{l}", 65, H * DO)
        for grp in range(NG):
            add(f"Wl{l}_{grp}", 32 * GSZ[grp], DLIN)
    for grp in range(NG):
        add(f"E{grp}", H, P)
        add(f"Sel{grp}", 32 * GSZ[grp], H)
    add("I64p", DLIN + 1, DLIN)
    return L, c


def _f32_layout():
    F = {}
    c = 0

    def add(name, rows, cols):
        nonlocal c
        F[name] = (rows, c, cols)
        c += cols

    add("Wf0", 2 * DLIN, 128)
    add("Wf1", 128, 64)
    add("Wf2", 64, 1)
    add("bl0", DLIN, 1)
    add("bl1", DLIN, 1)
    add("bf0", 128, 1)
    add("bf1", 64, 1)
    add("bf2", 1, 1)
    return F, c


BLOB_L, WCOLS = _blob_layout()
F32_L, FCOLS = _f32_layout()


def _build_nc():
    from contextlib import ExitStack

    import concourse.mybir as mybir
    import concourse.tile as tile
    from concourse import bacc

    f32 = mybir.dt.float32
    bf = mybir.dt.bfloat16
    AF = mybir.ActivationFunctionType
    ALU = mybir.AluOpType

    nc = bacc.Bacc()

    nf_d = nc.declare_dram_parameter("nf", [DIN + 1, G, N], bf, isOutput=False)
    adj_d = nc.declare_dram_parameter("adjT", [G, P, MC, N], bf, isOutput=False)
    aqoh_d = nc.declare_dram_parameter("aqoh", [P, G, 2, MC], bf, isOutput=False)
    aq32_d = nc.declare_dram_parameter("aq32", [P, G, MC], f32, isOutput=False)
    wb_d = nc.declare_dram_parameter("wb", [P, WCOLS], bf, isOutput=False)
    wf_d = nc.declare_dram_parameter("wf", [P, FCOLS], f32, isOutput=False)
    out_d = nc.declare_dram_parameter("out", [1, G], f32, isOutput=True)

    with tile.TileContext(nc) as tc, ExitStack() as ctx:
        wpool = ctx.enter_context(tc.tile_pool(name="w", bufs=1))
        gpool = ctx.enter_context(tc.tile_pool(name="g", bufs=2))
        work = ctx.enter_context(tc.tile_pool(name="work", bufs=3))
        misc = ctx.enter_context(tc.tile_pool(name="misc", bufs=2))
        persist = ctx.enter_context(tc.tile_pool(name="persist", bufs=1))
        ps_s = ctx.enter_context(tc.tile_pool(name="ps_s", bufs=2, space="PSUM"))
        ps_o = ctx.enter_context(tc.tile_pool(name="ps_o", bufs=1, space="PSUM"))
        ps_m = ctx.enter_context(tc.tile_pool(name="ps_m", bufs=1, space="PSUM"))

        # ---- DMAs (all issued up front; transfers overlap compute) ----
        wb_sb = wpool.tile([P, WCOLS], bf)
        nc.sync.dma_start(wb_sb[:], wb_d[:])
        wf_sb = wpool.tile([P, FCOLS], f32)
        nc.sync.dma_start(wf_sb[:], wf_d[:])
        nfT = wpool.tile([DIN + 1, G, N], bf)
        nc.sync.dma_start(nfT[:], nf_d[:])
        aqoh = wpool.tile([P, G, 2, MC], bf)
        nc.sync.dma_start(aqoh[:], aqoh_d[:])
        aq32 = wpool.tile([P, G, MC], f32)
        nc.sync.dma_start(aq32[:], aq32_d[:])
        adjb = wpool.tile([P, G, MC, N], bf)
        for g_ in range(G):
            nc.sync.dma_start(adjb[:, g_, :, :], adj_d[g_])

        W = {k: wb_sb[0:r, c:c + n] for k, (r, c, n) in BLOB_L.items()}
        F = {k: wf_sb[0:r, c:c + n] for k, (r, c, n) in F32_L.items()}

        feat_sb = persist.tile([P, G], f32)
        out_sb = persist.tile([1, G], f32)

        def elu(dst, src_ps, bias, p, f, dt, tg):
            # dst = elu(src+bias) = min(exp(src+bias)-1, 0) + max(src+bias, 0)
            # ACT does only the exp pass (it is the global bottleneck).
            e = work.tile([p, f], dt, tag=f"elu_e_{tg}", name=f"elu_e_{tg}")
            nc.scalar.activation(e[:], src_ps, AF.Exp, bias=bias)
            nc.vector.tensor_scalar(e[:], e[:], 1.0, 0.0, ALU.subtract, ALU.min)
            r = work.tile([p, f], dt, tag=f"elu_r_{tg}", name=f"elu_r_{tg}")
            if isinstance(bias, float):
                nc.vector.tensor_scalar(r[:], src_ps, bias, 0.0, ALU.add, ALU.max)
            else:
                nc.vector.tensor_scalar(r[:], src_ps, bias, 0.0, ALU.add, ALU.max)
            nc.vector.tensor_add(dst, e[:], r[:])

        # per-graph state
        x0a = [None] * G
        qkt = [[None] * NG for _ in range(G)]
        vsb = [None] * G
        o_ps_g = [None] * G
        x1a = [None] * G

        def chunksA(g):
            def c_x0():
                x0ps = ps_m.tile([DINIT, N], f32, tag="m", name="x0ps")
                nc.tensor.matmul(x0ps[:], W["Wi"][:], nfT[:, g, :],
                                 start=True, stop=True)
                xa = gpool.tile([DINIT + 1, N], bf, tag="x0a", name="x0a")
                elu(xa[0:DINIT, :], x0ps[:], 0.0, DINIT, N, bf, "x0")
                nc.vector.memset(xa[DINIT:DINIT + 1, :], 1.0)
                x0a[g] = xa

            def c_qk(grp):
                xa = x0a[g]
                sp = ps_s.tile([P, 2, N], f32, tag="s", name=f"qkp{grp}")
                nc.tensor.matmul(sp[:, 0, :], W[f"Wq0_{grp}"][:], xa[:],
                                 start=True, stop=True)
                nc.tensor.matmul(sp[:, 1, :], W[f"Wk0_{grp}"][:], xa[:],
                                 start=True, stop=True)
                qk = gpool.tile([P, 2, N], bf, tag=f"qk{grp}", name=f"qk{grp}")
                if grp == 1:
                    nc.scalar.copy(qk[:], sp[:])
                else:
                    nc.vector.tensor_copy(qk[:], sp[:])
                qkt[g][grp] = qk

            def c_v():
                xa = x0a[g]
                vps = ps_s.tile([P, MC, P], f32, tag="s", name="vps")
                for mc in range(MC):
                    nc.tensor.matmul(vps[:, mc, :], xa[:, mc * P:(mc + 1) * P],
                                     W["Wv0"][:], start=True, stop=True)
                vt = gpool.tile([P, MC, H, 32], bf, tag="vsb", name="vsb")
                nc.gpsimd.memset(vt[:], 0.0)
                nc.vector.memset(vt[:, :, :, DO:DO + 1], 1.0)
                for mc in range(MC):
                    nc.vector.tensor_copy(
                        vt[:, mc, :, 0:DO],
                        vps[:, mc, :].rearrange("p (h e) -> p h e", h=H))
                vsb[g] = vt

            return [c_x0, lambda: c_qk(0), lambda: c_qk(1), lambda: c_qk(2), c_v]

        def phaseB(g, stuffers=None):
            stuffers = stuffers or []
            o_ps = [ps_o.tile([P, N], f32, tag=f"o{grp}", name=f"o{grp}")
                    for grp in range(NG)]
            o_ps_g[g] = o_ps
            iters = [(pr, mc) for pr in PAIRS for mc in range(MC)]
            pend = None
            for idx in range(len(iters) + 1):
                if idx < len(iters):
                    (ha, hb), mc = iters[idx]
                    sp = ps_s.tile([P, 2, N], f32, tag="s", name="sp")
                    for r, h in enumerate((ha, hb)):
                        grp, pos = HMAP[h]
                        nc.tensor.matmul(
                            sp[:, r, :],
                            qkt[g][grp][32 * pos:32 * pos + DH, 1, mc * P:(mc + 1) * P],
                            qkt[g][grp][32 * pos:32 * pos + DH, 0, :],
                            start=True, stop=True)
                    ex = work.tile([P, 2, N], bf, tag="ex")
                    nc.scalar.activation(ex[:], sp[:], AF.Exp)
                    pm = work.tile([P, 2, N], bf, tag="pm")
                    nc.vector.tensor_tensor(
                        pm[:], ex[:],
                        adjb[:, g, mc, None, :].to_broadcast((P, 2, N)),
                        ALU.mult)
                    cur = ((ha, hb), mc, pm)
                else:
                    cur = None
                if pend is not None:
                    (ha, hb), mc, pmp = pend
                    for r, h in enumerate((ha, hb)):
                        grp, pos = HMAP[h]
                        nc.tensor.matmul(
                            o_ps[grp][32 * pos:32 * pos + 32, :],
                            vsb[g][:, mc, h, :], pmp[:, r, :],
                            start=(mc == 0), stop=(mc == MC - 1))
                pend = cur
                if stuffers:
                    stuffers.pop(0)()
            while stuffers:
                stuffers.pop(0)()

        def chunksC(g):
            osb = []
            scrs = []
            rec_box = []

            def c_osb():
                o_ps = o_ps_g[g]
                for grp in range(NG):
                    rg = 32 * GSZ[grp]
                    t = gpool.tile([rg, N], bf, tag=f"osb{grp}", name=f"osb{grp}")
                    if grp == 1:
                        nc.scalar.copy(t[:], o_ps[grp][0:rg, :])
                    else:
                        nc.vector.tensor_copy(t[:], o_ps[grp][0:rg, :])
                    osb.append(t)

            def c_den():
                den = ps_m.tile([H, N], f32, tag="m", name="den")
                for grp in range(NG):
                    nc.tensor.matmul(den[:], W[f"Sel{grp}"][:], osb[grp][:],
                                     start=(grp == 0), stop=(grp == NG - 1))
                rec = misc.tile([H, N], bf, tag="rec")
                with nc.allow_low_precision(reason="softmax denom O(1); bf16 ok"):
                    nc.vector.reciprocal(rec[:], den[:])
                rec_box.append(rec)

            def c_scr(grp):
                rg = 32 * GSZ[grp]
                dps = ps_m.tile([P, N], f32, tag="m", name="dps")
                nc.tensor.matmul(dps[:], W[f"E{grp}"][:], rec_box[0][:],
                                 start=True, stop=True)
                dsb = misc.tile([P, N], bf, tag="dsb")
                nc.vector.tensor_copy(dsb[:], dps[:])
                scr = misc.tile([rg, N], bf, tag=f"scr{grp}", name=f"scr{grp}")
                nc.vector.tensor_tensor(scr[:], osb[grp][:], dsb[0:rg, :], ALU.mult)
                scrs.append(scr)

            def c_x1():
                x1ps = ps_m.tile([DLIN, N], f32, tag="m", name="x1ps")
                for grp in range(NG):
                    nc.tensor.matmul(x1ps[:], W[f"Wl0_{grp}"][:], scrs[grp][:],
                                     start=(grp == 0), stop=(grp == NG - 1))
                xa = gpool.tile([DLIN + 1, N], bf, tag="x1a", name="x1a")
                elu(xa[0:DLIN, :], x1ps[:], F["bl0"][:], DLIN, N, bf, "x1")
                nc.vector.memset(xa[DLIN:DLIN + 1, :], 1.0)
                x1a[g] = xa

            return [c_osb, c_den, lambda: c_scr(0), lambda: c_scr(1),
                    lambda: c_scr(2), c_x1]

        def chunksD(g):
            st = {}

            def c_gather():
                xa = x1a[g]
                ndp = ps_m.tile([P, MC, DLIN], f32, tag="m", name="ndp")
                for mc in range(MC):
                    nc.tensor.matmul(ndp[:, mc, :], xa[:, mc * P:(mc + 1) * P],
                                     W["I64p"][:], start=True, stop=True)
                x1nd = gpool.tile([P, MC, DLIN], bf, tag="x1nd", bufs=1, name="x1nd")
                nc.vector.tensor_copy(x1nd[:], ndp[:])
                x1qps = ps_m.tile([DLIN, 1], f32, tag="m", name="x1qps")
                for mc in range(MC):
                    nc.tensor.matmul(x1qps[:], x1nd[:, mc, :],
                                     aqoh[:, g, 1, mc:mc + 1],
                                     start=(mc == 0), stop=(mc == MC - 1))
                nc.vector.tensor_copy(feat_sb[0:DLIN, g:g + 1], x1qps[:])
                x1qa = gpool.tile([DLIN + 1, 1], bf, tag="x1qa", bufs=1, name="x1qa")
                nc.vector.tensor_copy(x1qa[0:DLIN, :], x1qps[:])
                nc.vector.memset(x1qa[DLIN:DLIN + 1, :], 1.0)
                st["x1qa"] = x1qa

            def c_q2():
                q2ps = ps_m.tile([P, NG], f32, tag="m", name="q2ps")
                for grp in range(NG):
                    nc.tensor.matmul(q2ps[:, grp:grp + 1], W[f"Wq1_{grp}"][:],
                                     st["x1qa"][:], start=True, stop=True)
                q2bd = []
                for grp in range(NG):
                    qb = gpool.tile([P, 3], bf, tag=f"q2bd{grp}", bufs=1,
                                    name=f"q2bd{grp}")
                    nc.vector.memset(qb[:], 0.0)
                    for pos in range(GSZ[grp]):
                        nc.vector.tensor_copy(
                            qb[32 * pos:32 * pos + DH, pos:pos + 1],
                            q2ps[32 * pos:32 * pos + DH, grp:grp + 1])
                    q2bd.append(qb)
                st["q2bd"] = q2bd

            def c_k2(grp):
                k2ps = ps_s.tile([P, N], f32, tag="s", name=f"k2ps{grp}")
                nc.tensor.matmul(k2ps[:], W[f"Wk1_{grp}"][:], x1a[g][:],
                                 start=True, stop=True)
                kb = gpool.tile([P, N], bf, tag=f"k2t{grp}", bufs=1, name=f"k2t{grp}")
                if grp == 1:
                    nc.scalar.copy(kb[:], k2ps[:])
                else:
                    nc.vector.tensor_copy(kb[:], k2ps[:])
                st.setdefault("k2t", {})[grp] = kb

            def c_v2():
                xa = x1a[g]
                v2ps = ps_s.tile([P, MC, P], f32, tag="s", name="v2ps")
                for mc in range(MC):
                    nc.tensor.matmul(v2ps[:, mc, :], xa[:, mc * P:(mc + 1) * P],
                                     W["Wv1"][:], start=True, stop=True)
                v2 = gpool.tile([P, MC, H, 32], bf, tag="v2sb", bufs=1, name="v2sb")
                nc.gpsimd.memset(v2[:], 0.0)
                nc.vector.memset(v2[:, :, :, DO:DO + 1], 1.0)
                for mc in range(MC):
                    nc.vector.tensor_copy(
                        v2[:, mc, :, 0:DO],
                        v2ps[:, mc, :].rearrange("p (h e) -> p h e", h=H))
                    # fold query-row adjacency mask (incl. ones col)
                    nc.vector.tensor_scalar_mul(
                        v2[:, mc, :, :], v2[:, mc, :, :], aq32[:, g, mc:mc + 1])
                st["v2"] = v2

            def c_att(mc):
                if mc == 0:
                    st["o2p"] = ps_m.tile([P, NG * MC], f32, tag="m", name="o2p")
                k2t, q2bd, v2, o2p = st["k2t"], st["q2bd"], st["v2"], st["o2p"]
                s2p = ps_s.tile([P, H], f32, tag="s", name="s2p")
                for grp in range(NG):
                    nc.tensor.matmul(s2p[:, 3 * grp:3 * grp + GSZ[grp]],
                                     k2t[grp][:, mc * P:(mc + 1) * P],
                                     q2bd[grp][:, 0:GSZ[grp]],
                                     start=True, stop=True)
                ex2 = misc.tile([P, H], bf, tag="ex2")
                nc.scalar.activation(ex2[:], s2p[:], AF.Exp)
                for grp in range(NG):
                    for pos in range(GSZ[grp]):
                        h = 3 * grp + pos
                        nc.tensor.matmul(o2p[32 * pos:32 * pos + 32,
                                             grp * MC + mc:grp * MC + mc + 1],
                                         v2[:, mc, h, :], ex2[:, h:h + 1],
                                         start=True, stop=True)

            def c_fin():
                o2p = st["o2p"]
                o2f = misc.tile([P, NG], f32, tag="o2f")
                nc.vector.tensor_reduce(o2f[:],
                                        o2p.rearrange("p (g m) -> p g m", g=NG),
                                        mybir.AxisListType.X, ALU.add)
                o2b = misc.tile([P, NG], bf, tag="o2b")
                nc.vector.tensor_copy(o2b[:], o2f[:])
                den2 = ps_m.tile([H, 1], f32, tag="m", name="den2")
                for grp in range(NG):
                    nc.tensor.matmul(den2[:], W[f"Sel{grp}"][:],
                                     o2b[0:32 * GSZ[grp], grp:grp + 1],
                                     start=(grp == 0), stop=(grp == NG - 1))
                rec2 = misc.tile([H, 1], bf, tag="rec2")
                with nc.allow_low_precision(reason="softmax denom O(1); bf16 ok"):
                    nc.vector.reciprocal(rec2[:], den2[:])
                d2ps = ps_m.tile([P, NG], f32, tag="m", name="d2ps")
                for grp in range(NG):
                    nc.tensor.matmul(d2ps[:, grp:grp + 1], W[f"E{grp}"][:], rec2[:],
                                     start=True, stop=True)
                d2f = misc.tile([P, NG], f32, tag="d2f")
                nc.vector.tensor_copy(d2f[:], d2ps[:])
                scr2s = []
                for grp in range(NG):
                    rg = 32 * GSZ[grp]
                    scr2 = misc.tile([rg, 1], bf, tag=f"scr2_{grp}",
                                     name=f"scr2_{grp}")
                    nc.vector.tensor_tensor(scr2[:], o2f[0:rg, grp:grp + 1],
                                            d2f[0:rg, grp:grp + 1], ALU.mult)
                    scr2s.append(scr2)
                x2ps = ps_m.tile([DLIN, 1], f32, tag="m", name="x2ps")
                for grp in range(NG):
                    nc.tensor.matmul(x2ps[:], W[f"Wl1_{grp}"][:], scr2s[grp][:],
                                     start=(grp == 0), stop=(grp == NG - 1))
                elu(feat_sb[DLIN:2 * DLIN, g:g + 1], x2ps[:], F["bl1"][:],
                    DLIN, 1, f32, "x2")

            return [c_gather, c_q2, lambda: c_k2(0), lambda: c_k2(1),
                    lambda: c_k2(2), c_v2, lambda: c_att(0), lambda: c_att(1),
                    lambda: c_att(2), lambda: c_att(3), c_fin]

        # ---- emission schedule ----
        # B(g) hosts stuffed chunks: C(g-1), D(g-1), A(g+1); C/D of the last
        # graph run exposed at the tail.
        for c in chunksA(0):
            c()
        phaseB(0, chunksA(1))
        for g in range(1, G):
            stuff = chunksC(g - 1) + chunksD(g - 1)
            if g + 1 < G:
                stuff += chunksA(g + 1)
            phaseB(g, stuff)
        for c in chunksC(G - 1) + chunksD(G - 1):
            c()

        # ---- MLP head over all graphs (fp32, tiny) ----
        h1ps = ps_m.tile([128, G], f32, tag="m", name="h1ps")
        nc.tensor.matmul(h1ps[:], F["Wf0"][:], feat_sb[:], start=True, stop=True)
        h1 = persist.tile([128, G], f32, tag="h1")
        elu(h1[:], h1ps[:], F["bf0"][:], 128, G, f32, "m1")
        h2ps = ps_m.tile([64, G], f32, tag="m", name="h2ps")
        nc.tensor.matmul(h2ps[:], F["Wf1"][:], h1[:], start=True, stop=True)
        h2 = persist.tile([64, G], f32, tag="h2")
        elu(h2[:], h2ps[:], F["bf1"][:], 64, G, f32, "m2")
        h3ps = ps_m.tile([1, G], f32, tag="m", name="h3ps")
        nc.tensor.matmul(h3ps[:], F["Wf2"][:], h2[:], start=True, stop=True)
        elu(out_sb[:], h3ps[:], F["bf2"][:], 1, G, f32, "m3")
        nc.vector.tensor_scalar_mul(out_sb[:], out_sb[:], float(SCALE))
        nc.sync.dma_start(out_d[:], out_sb[:])

    nc.compile()
    return nc


def _prep_weights(inputs):
    f32 = np.float32
    vals = {}

    def aug(Wm, bv):
        return np.concatenate([np.asarray(Wm, f32).reshape(Wm.shape[0], -1),
                               np.asarray(bv, f32).reshape(1, -1)], axis=0)

    def pad3(Wa):  # [d, H*DH] -> NG x [d, 128] group-padded (32-stride)
        outs = []
        for grp in range(NG):
            Om = np.zeros((Wa.shape[0], P), f32)
            for pos in range(GSZ[grp]):
                h = 3 * grp + pos
                Om[:, 32 * pos:32 * pos + DH] = Wa[:, DH * h:DH * (h + 1)]
            outs.append(Om)
        return outs

    vals["Wi"] = aug(inputs["W_init"], inputs["b_init"])
    for l in range(2):
        s = 1.0 / np.sqrt(DH)
        for grp, Om in enumerate(pad3(aug(inputs[f"Wq{l}"], inputs[f"bq{l}"]))):
            vals[f"Wq{l}_{grp}"] = Om
        for grp, Om in enumerate(pad3(aug(np.asarray(inputs[f"Wk{l}"], f32) * s,
                                          np.asarray(inputs[f"bk{l}"], f32) * s))):
            vals[f"Wk{l}_{grp}"] = Om
        vals[f"Wv{l}"] = aug(inputs[f"Wv{l}"], inputs[f"bv{l}"])
        Wl = np.asarray(inputs[f"Wl{l}"], f32)  # [H*DO, DLIN]
        for grp in range(NG):
            Wlp = np.zeros((32 * GSZ[grp], DLIN), f32)
            for pos in range(GSZ[grp]):
                h = 3 * grp + pos
                Wlp[32 * pos:32 * pos + DO] = Wl[DO * h:DO * (h + 1)]
            vals[f"Wl{l}_{grp}"] = Wlp
    for grp in range(NG):
        E = np.zeros((H, P), f32)
        Sel = np.zeros((32 * GSZ[grp], H), f32)
        for pos in range(GSZ[grp]):
            E[3 * grp + pos, 32 * pos:32 * pos + DO + 1] = 1.0
            Sel[32 * pos + DO, 3 * grp + pos] = 1.0
        vals[f"E{grp}"] = E
        vals[f"Sel{grp}"] = Sel
    vals["I64p"] = np.concatenate([np.eye(DLIN, dtype=f32),
                                   np.zeros((1, DLIN), f32)], axis=0)

    wb = np.zeros((P, WCOLS), f32)
    for k, (r, c, n) in BLOB_L.items():
        wb[0:r, c:c + n] = vals[k]

    fvals = {
        "Wf0": np.asarray(inputs["Wf0"], f32),
        "Wf1": np.asarray(inputs["Wf1"], f32),
        "Wf2": np.asarray(inputs["Wf2"], f32),
        "bl0": np.asarray(inputs["bl0"], f32).reshape(DLIN, 1),
        "bl1": np.asarray(inputs["bl1"], f32).reshape(DLIN, 1),
        "bf0": np.asarray(inputs["bf0"], f32).reshape(128, 1),
        "bf1": np.asarray(inputs["bf1"], f32).reshape(64, 1),
        "bf2": np.asarray(inputs["bf2"], f32).reshape(1, 1),
    }
    wf = np.zeros((P, FCOLS), f32)
    for k, (r, c, n) in F32_L.items():
        wf[0:r, c:c + n] = fvals[k]
    return {"wb": wb.astype(BF), "wf": wf}


def _prep_core_inputs(inputs, core):
    """Marshal one core's shard (graphs core*G .. core*G+G-1) into device layout."""
    f32 = np.float32
    sl = slice(core * G, (core + 1) * G)
    nf = np.asarray(inputs["node_features"], f32)[sl]     # [G, N, DIN]
    adj = np.asarray(inputs["adj"], f32)[sl]              # [G, N, N]
    masks = np.asarray(inputs["masks"], f32)[sl]          # [G, N]
    qidx = np.asarray(inputs["query_idxs"])[sl]           # [G]

    nfT = np.ones((DIN + 1, G, N), f32)
    nfT[0:DIN] = np.transpose(nf, (2, 0, 1))

    adjm = ((np.transpose(adj, (0, 2, 1)) > 0) & (masks[:, :, None] > 0)).astype(f32)
    # [G, N(m), N(n)] -> [G, 128(p), MC, N]: row mc*128+p -> (p, mc)
    adjdev = adjm.reshape(G, MC, P, N).transpose(0, 2, 1, 3)

    aqoh = np.zeros((P, G, 2, MC), f32)
    for g in range(G):
        aq = ((adj[g, qidx[g]] > 0) & (masks[g] > 0)).astype(f32)
        aqoh[:, g, 0, :] = aq.reshape(MC, P).T
        oh = np.zeros(N, f32)
        oh[qidx[g]] = 1.0
        aqoh[:, g, 1, :] = oh.reshape(MC, P).T
    return {
        "nf": np.ascontiguousarray(nfT).astype(BF),
        "adjT": np.ascontiguousarray(adjdev).astype(BF),
        "aqoh": np.ascontiguousarray(aqoh).astype(BF),
        "aq32": np.ascontiguousarray(aqoh[:, :, 0, :]),
    }


def kernel(**inputs) -> np.ndarray:
    from concourse.bass_utils import run_bass_kernel_spmd

    nc = _build_nc()
    w = _prep_weights(inputs)
    in_maps = []
    for core in range(NCORES):
        m = _prep_core_inputs(inputs, core)
        m.update(w)
        in_maps.append(m)
    res = run_bass_kernel_spmd(nc, in_maps, list(range(NCORES)))
    out = np.concatenate([res.results[i]["out"][0] for i in range(NCORES)])
    return out.astype(np.float32).reshape(B, 1)
